# revision 11
# baseline (speedup 1.0000x reference)
"""Hanning template layer for TRN2: weighted sum of 4 Hanning correlations
== single 80-tap correlation.  out[b,j] = sum_i c[i] * x[b, j+i-40].

Device scheme (per core, 8 batch rows of L=65536, pure data parallel):
  Host ships x as fp16, each row padded to 528*128 with a 64-sample
  zero lead-in:  xrow[64:64+L] = x[b].
  1. DMA-xbar-transpose loads xt[k, v] = xrow[128 v + k]   [128, 528/row]
     (no PE transposes, no PSUM round trip for layout).
  2. conv as 8 banded matmuls per row with the SIGNAL stationary:
       Y[128 Q + n', m] = sum_{s=0,1} sum_k xt[k, 128 Q + n' + s] * Bs[s][k, m]
       Bs[s][k, m] = c[128 s + k - m - 24]  (banded Toeplitz, fp16)
     The 64-sample lead-in centers the 80-tap window so TWO shifted
     matmuls (not three) cover every output: stationary = xt column
     slice, moving = band matrix, PSUM accumulates f32.  Output lands
     directly in natural row-major layout (partition = 128-sample
     block index) -- no transpose back.
  3. DVE/ACT copy PSUM->SBUF, per-row DMA out (512 B runs).

Constraints baked in (learned on HW):
  - walrus codegen allows only ONE sync wait per instruction -> a post-
    pass splits residual multi-waits onto cloned per-engine Drains.
  - DMA transpose: 16-bit dtype only, src rows %16 == 0, cols %128 == 0.
"""

import copy as _copy

import numpy as np

import concourse.bass as bass
import concourse.mybir as mybir
from concourse.tile import TileContext
from concourse.bass_utils import run_bass_kernel_spmd

B, L = 64, 65536
N_CORES = 8
ROWS = B // N_CORES          # 8 rows per core
P = 128                      # partitions / block size
NBLK = L // P                # 512 blocks of 128 samples per row
VB = 528                     # padded blocks per row (mult of 16 for xbar)
OFF = 64                     # zero lead-in samples (centers the window)
TAPS = 80
HALF = 40
NSH = 2                      # shifted matmuls per output chunk
NCH = 4                      # output chunks of 128 blocks per row

F32 = mybir.dt.float32
F16 = mybir.dt.float16

WIDTHS = [10, 20, 30, 40]


def _combined_filter(template_weights: np.ndarray) -> np.ndarray:
    """softmax-weighted sum of hanning(2w) templates aligned at offset d=-40."""
    w = template_weights.astype(np.float64)
    e = np.exp(w - w.max())
    sm = e / e.sum()
    c = np.zeros(TAPS, dtype=np.float64)
    for t, wd in enumerate(WIDTHS):
        h = np.hanning(2 * wd)
        # contributes at filter index i = d + 40 for d in [-wd, wd)
        c[HALF - wd : HALF + wd] += sm[t] * h
    return c


def _band_matrices(c: np.ndarray) -> np.ndarray:
    """Bs[s][k, m] = c[128 s + k - m - 24] where in range, else 0."""
    Bs = np.zeros((NSH, P, P), dtype=np.float64)
    k = np.arange(P)[:, None]
    m = np.arange(P)[None, :]
    for s in range(NSH):
        i = 128 * s + k - m - 24
        ok = (i >= 0) & (i < TAPS)
        Bs[s][ok] = c[i[ok]]
    return Bs


def _split_excess_waits(nc, limit=1):
    """Move excess sync waits onto cloned same-engine Drain instructions
    (walrus codegen rejects >1 wait per instruction)."""
    drain_tmpl = {}
    for func in nc.m.functions:
        for bb in func.blocks:
            for inst in bb.instructions:
                if inst.opcode == "Drain" and inst.engine not in drain_tmpl:
                    drain_tmpl[inst.engine] = inst
    for func in nc.m.functions:
        for bb in func.blocks:
            changed = False
            out = []
            for inst in bb.instructions:
                si = inst.sync_info
                if si and len(si.on_wait) > limit:
                    waits = list(si.on_wait)
                    keep, extra = waits[-limit:], waits[:-limit]
                    tmpl = inst if inst.opcode == "Drain" else drain_tmpl.get(inst.engine)
                    assert tmpl is not None, (
                        f"no drain template for engine {inst.engine} ({inst.opcode})"
                    )
                    for j in range(0, len(extra), limit):
                        cln = _copy.deepcopy(tmpl)
                        cln.name = f"{inst.name}w{j}"
                        cln.engine = inst.engine
                        csi = cln.sync_info
                        csi.on_wait = extra[j : j + limit]
                        csi.on_update = []
                        cln.sync_info = csi
                        out.append(cln)
                        changed = True
                    si.on_wait = keep
                    inst.sync_info = si
                out.append(inst)
            if changed:
                bb.instructions = out


def build_nc():
    nc = bass.Bass()
    # pre-transposed on host: x[r, k, v] = xrow_r[128 v + k]
    x = nc.dram_tensor("x", [ROWS, P, VB], F16, kind="ExternalInput")
    consts = nc.dram_tensor("consts", [P, NSH * P], F16, kind="ExternalInput")
    # transposed output layout: y[r, m, n] = y_nat[r, 128 n + m]
    # -> each partition writes one contiguous 1 KiB run per row (fp16);
    #    host un-shuffles (cheap numpy transpose).
    y = nc.dram_tensor("y", [ROWS, P, NBLK], F16, kind="ExternalOutput")

    with TileContext(nc) as tc:
        with (
            tc.tile_pool(name="sbuf", bufs=4) as pool,
            tc.tile_pool(name="opool", bufs=4) as opool,
            tc.tile_pool(name="cpool", bufs=1) as cpool,
            tc.tile_pool(name="psum", bufs=4, space="PSUM") as pp,
            tc.tile_pool(name="wpsum", bufs=1, space="PSUM") as wp,
        ):
            cst = cpool.tile([P, NSH * P], F16)
            nc.sync.dma_start(out=cst, in_=consts[:, :])

            # HAM warm-up: dummy matmuls on zeros while the first x rows are
            # still in flight (PE would idle; this flips the clock gate to
            # 8/8 so the real matmuls run at 2.4 GHz).
            wtile = cpool.tile([P, NBLK], F16)
            nc.vector.memset(wtile, 0.0)
            ps_w = wp.tile([P, NBLK], F32)
            for _ in range(7):
                nc.tensor.matmul(ps_w, wtile[:, 0:P], wtile, start=True, stop=True)

            for r in range(ROWS):
                # xt[k, v] = x[r, k, v]  (transposed on host)
                xt = pool.tile([P, VB], F16, tag="xt")
                ld_eng = nc.sync if r % 2 == 0 else nc.scalar
                ld_eng.dma_start(out=xt, in_=x[r])

                # OT[m, n] = y[128 n + m] = sum_s sum_k Bs[s][k, m] xt'[k, n+s]
                ps = pp.tile([P, NBLK], F32, tag="ps")
                for s in range(NSH):
                    nc.tensor.matmul(
                        ps,
                        cst[:, P * s : P * (s + 1)],
                        xt[:, s : s + NBLK],
                        start=(s == 0),
                        stop=(s == NSH - 1),
                    )
                # copy halves on both engines in parallel
                osb = opool.tile([P, NBLK], F16, tag="osb")
                half = NBLK // 2
                nc.vector.tensor_copy(out=osb[:, 0:half], in_=ps[:, 0:half])
                nc.scalar.copy(out=osb[:, half:], in_=ps[:, half:])
                st_eng = nc.scalar if r % 2 == 0 else nc.sync
                st_eng.dma_start(out=y[r], in_=osb)

    _split_excess_waits(nc)
    return nc


_NC_CACHE = None


def _host_prep(x: np.ndarray, template_weights: np.ndarray):
    """fp16 cast + pad + block-transpose of x rows; fp16 band-matrix consts."""
    c = _combined_filter(np.asarray(template_weights, dtype=np.float32))
    Bs = _band_matrices(c)
    consts = np.concatenate(list(Bs), axis=1).astype(np.float16)

    xpad = np.zeros((B, VB * P), dtype=np.float16)
    xpad[:, OFF : OFF + L] = np.asarray(x, dtype=np.float32)
    # x_t[r, k, v] = xrow_r[128 v + k]
    x_t = np.ascontiguousarray(xpad.reshape(B, VB, P).transpose(0, 2, 1))
    return x_t, consts


def kernel(x: np.ndarray, template_weights: np.ndarray) -> np.ndarray:
    global _NC_CACHE
    xpad, consts = _host_prep(x, template_weights)

    if _NC_CACHE is None:
        _NC_CACHE = build_nc()
    nc = _NC_CACHE

    in_maps = [
        {"x": xpad[core * ROWS : (core + 1) * ROWS], "consts": consts}
        for core in range(N_CORES)
    ]
    res = run_bass_kernel_spmd(nc, in_maps, core_ids=list(range(N_CORES)))
    out = np.concatenate([r["y"] for r in res.results], axis=0)
    # un-shuffle transposed layout: y_nat[b, 128 n + m] = out[b, m, n]
    return np.ascontiguousarray(
        out.astype(np.float32).transpose(0, 2, 1)
    ).reshape(B, L)
